# revision 31
# baseline (speedup 1.0000x reference)
"""Trainium2 Bass kernel for nn_CrossAttention (B=2, S=2048, E=1024, H=16, d=64).

Sharding: 8 cores = 2 batches x 4 query-blocks of 512 rows. Each core gets its
query block + the full values[b] for its batch; no collectives needed.

Algebra (host-folded): with q = q_in@Wq.T+bq, v = v_in@Wv.T+bv, k = v@Wk.T+bk:
  scores = q @ k.T = qe @ v_in.T + const(q-row)   [softmax-shift-invariant]
    where qe = q_in @ (Wq.T@Wk@Wv) + bq@Wk@Wv
  out    = attn @ v @ Wd.T + bd = (attn @ v_in) @ (Wd@blockdiag(Wv)).T
           + (bd + Wd@tile(bv))                   [attn rows sum to 1]
So the device never computes k or v projections.

Per-core device pipeline (feature-major / transposed layouts):
  1. qeT = WQ2 @ q_inT + cq2 per head-pair (WQ2 = blkdiag pair of fold)
  2. scores.T = v_inT.T-slices @ qeT   (row-tiled 2-head concurrent matmuls)
  3. E = exp(scores.T * 0.125): split between ACT (exact spline) and DVE
     (Schraudolph: bf16-bits = int16(x*A + B), ~3.3% max rel err)
  4. U.T = [v_in | ones].T @ E  (ones col yields softmax denom as row 64)
  5. out.T = U.T * (1/r) via PE-replicated reciprocal rows
  6. final = out.T-slices @ WdT' + bd'  -> natural layout -> DMA

Software pipeline: slot p computes scores(p) and U(p-1); the exp engines
(ACT + DVE split) pace the steady state. Reciprocals are batched (the DVE
reciprocal uop costs ~3.3us regardless of partition count): pairs 0-6 in one
op at slot-8 start, pair 7 in the tail. Slot 8 also runs the per-pair
normalize and output-projection m-blocks 0/1 (kk<=4) to shorten the tail.
"""

import sys

for _p in ("/opt/trn_rl_repo",):
    if _p not in sys.path:
        sys.path.insert(0, _p)

from contextlib import ExitStack

import ml_dtypes
import numpy as np

import concourse.bass as bass
import concourse.tile as tile
from concourse import bacc, mybir
from concourse.bass_utils import run_bass_kernel_spmd

F32 = mybir.dt.float32
BF16 = mybir.dt.bfloat16
F32R = mybir.dt.float32r
I16 = mybir.dt.int16
EXP = mybir.ActivationFunctionType.Exp
MULT = mybir.AluOpType.mult
ADD = mybir.AluOpType.add

B, S, E, H, D = 2, 2048, 1024, 16, 64
N_CORES = 8
SQB = S * B // N_CORES  # 512 query rows per core
NP_BF16 = ml_dtypes.bfloat16

# Schraudolph fast-exp constants: bf16bits(exp(x/8)) ~= int16(x*SCH_A + SCH_B)
LOG2E = 1.4426950408889634
SCH_A = 0.125 * 128.0 * LOG2E
SCH_B = 16256.0 - 5.5027  # HW rounds-to-nearest (probe-verified)

# per pair: psB tiles with g < ACT_B go to ACT, rest to DVE Schraudolph
ACT_B = 1

_CACHE = {}


def _build_program():
    nc = bacc.Bacc("TRN2", target_bir_lowering=False, debug=False, num_devices=N_CORES)

    qT_in = nc.dram_tensor("qT_in", [E, SQB], BF16, kind="ExternalInput").ap()
    vT_in = nc.dram_tensor("vT_in", [E, S], BF16, kind="ExternalInput").ap()
    vnat_in = nc.dram_tensor("vnat_in", [S, H * 128], BF16, kind="ExternalInput").ap()
    wq2 = nc.dram_tensor("wq2", [128, 128], BF16, kind="ExternalInput").ap()
    cq2 = nc.dram_tensor("cq2", [128, 1], F32, kind="ExternalInput").ap()
    sel14 = nc.dram_tensor("sel14", [14, 7 * 128], F32R, kind="ExternalInput").ap()
    sel2 = nc.dram_tensor("sel2", [2, 128], F32R, kind="ExternalInput").ap()
    wdT = nc.dram_tensor("wdT", [E, E], BF16, kind="ExternalInput").ap()
    bd_rep = nc.dram_tensor("bd_rep", [128, E], F32, kind="ExternalInput").ap()
    out = nc.dram_tensor("out", [SQB, E], BF16, kind="ExternalOutput").ap()

    with tile.TileContext(nc) as tc, ExitStack() as ctx:
        # ---- pools ----
        wpool = ctx.enter_context(tc.tile_pool(name="w", bufs=1))
        ep = ctx.enter_context(tc.tile_pool(name="ep", bufs=20))
        u2p = ctx.enter_context(tc.tile_pool(name="u2", bufs=1))
        outp = ctx.enter_context(tc.tile_pool(name="outp", bufs=1))
        osbp = ctx.enter_context(tc.tile_pool(name="osb", bufs=2))
        sc_ps = ctx.enter_context(tc.tile_pool(name="scps", bufs=3, space="PSUM"))
        u_ps = ctx.enter_context(tc.tile_pool(name="ups", bufs=2, space="PSUM"))

        # ---- constants / persistent inputs ----
        # DMA order = need order: qe inputs, then per-pair v data, then
        # normalize/output-projection constants (needed only at the tail)
        wq2_s = wpool.tile([128, 128], BF16, tag="wq2")
        nc.sync.dma_start(wq2_s[:], wq2[:])
        cq2_s = wpool.tile([128, 1], F32, tag="cq2")
        nc.sync.dma_start(cq2_s[:], cq2[:])
        qin = []
        for p in range(8):
            t = wpool.tile([128, SQB], BF16, tag=f"qin{p}")
            nc.sync.dma_start(t[:], qT_in[p * 128 : (p + 1) * 128, :])
            qin.append(t)
        # feature-major values, one persistent tile per head-pair; vt[0]
        # first (needed at slot 0), then natural values (needed at slot 1),
        # then the remaining vt pair tiles (needed one per ~13us slot)
        vt = [None] * 8
        def load_vt(p):
            t = wpool.tile([128, S], BF16, tag=f"vt{p}", name=f"vt{p}")
            nc.sync.dma_start(t[:], vT_in[p * 128 : (p + 1) * 128, :])
            vt[p] = t
        load_vt(0)
        vna = []
        for tch in range(16):
            t = wpool.tile([128, H * 128], BF16, tag=f"vna{tch}")
            nc.sync.dma_start(t[:], vnat_in[tch * 128 : (tch + 1) * 128, :])
            vna.append(t)
        for p in range(1, 8):
            load_vt(p)
        sel14_s = wpool.tile([14, 7 * 128], F32R, tag="sel14")
        nc.sync.dma_start(sel14_s[:], sel14[:])
        sel2_s = wpool.tile([2, 128], F32R, tag="sel2")
        nc.sync.dma_start(sel2_s[:], sel2[:])
        bd_s = wpool.tile([128, E], F32, tag="bd")
        nc.sync.dma_start(bd_s[:], bd_rep[:])
        wd_s = []
        for kk in range(8):
            t = wpool.tile([128, E], BF16, tag=f"wd{kk}")
            nc.sync.dma_start(t[:], wdT[kk * 128 : (kk + 1) * 128, :])
            wd_s.append(t)

        # ---- qe projection: all pairs up-front (2 pairs per PSUM tile) ----
        qe_sb = wpool.tile([128, 8 * SQB], BF16, tag="qe")
        for pp in range(4):
            ps = sc_ps.tile([128, 1024], F32, tag="scps")
            for i in range(2):
                p = 2 * pp + i
                nc.tensor.matmul(
                    ps[:, i * 512 : (i + 1) * 512], wq2_s[:], qin[p][:],
                    start=True, stop=True,
                )
            nc.vector.tensor_scalar(
                qe_sb[:, pp * 1024 : (pp + 1) * 1024], ps[:], cq2_s[:], None,
                op0=ADD,
            )

        rg14 = wpool.tile([14, SQB], BF16, tag="rg14")   # denoms, pairs 0-6
        rr14 = wpool.tile([14, SQB], F32R, tag="rr14")
        rg7 = wpool.tile([2, SQB], BF16, tag="rg7")      # denoms, pair 7
        rr7 = wpool.tile([2, SQB], F32R, tag="rr7")
        U2 = [
            u2p.tile([128, SQB], BF16, tag=f"u2_{p}", name=f"u2_{p}")
            for p in range(8)
        ]
        outT = [None] * 8
        oproj_ps = [None] * 4

        def u_mm(pair, ups2, Elist, t):
            # one U chunk for both heads: E chunk t tile has h0 in cols
            # 0:512, h1 in cols 512:1024
            for h2, ups in ((0, ups2[0]), (1, ups2[1])):
                h = 2 * pair + h2
                nc.tensor.matmul(
                    ups[:], vna[t][:, h * 128 : (h + 1) * 128],
                    Elist[t][:, h2 * 512 : (h2 + 1) * 512],
                    start=(t == 0), stop=(t == 15),
                )

        def evac(pair, ups2):
            # evacuate U(pair) incl. denominator row 64 with ONE cast per
            # head; DMA (no engine time) then splits body and denominator
            for h2, ups in ((0, ups2[0]), (1, ups2[1])):
                h = 2 * pair + h2
                uh = osbp.tile([65, SQB], BF16, tag="uh", name=f"uh{h}")
                nc.vector.tensor_copy(uh[:], ups[0:65, :])
                nc.sync.dma_start(
                    U2[pair][h2 * 64 : (h2 + 1) * 64, :], uh[0:64, :]
                )
                if pair < 7:
                    nc.sync.dma_start(rg14[h : h + 1, :], uh[64:65, :])
                else:
                    nc.sync.dma_start(rg7[h2 : h2 + 1, :], uh[64:65, :])

        # ---- software-pipelined main loop: slot p = scores(p) + U(p-1) ----
        Eprev = None
        ups_prev = None
        for p in range(9):
            qe = qe_sb[:, p * SQB : (p + 1) * SQB] if p < 8 else None
            Ech = []
            if p > 0:
                ups_cur = (
                    u_ps.tile([128, 512], F32, tag="ups", name=f"upsA{p-1}"),
                    u_ps.tile([128, 512], F32, tag="ups", name=f"upsB{p-1}"),
                )
            if p == 8:
                # batched reciprocal for pairs 0-6; first in slot 8's DVE
                # queue so the sel matmuls below never wait on it
                with nc.allow_low_precision(reason="f32r full fp32 range; f22 mantissa ok for softmax denom"):
                    nc.vector.reciprocal(rr14[:], rg14[:])
                oproj_ps[0] = sc_ps.tile([128, 1024], F32, tag="scps", name="oproj0")
                oproj_ps[1] = sc_ps.tile([128, 1024], F32, tag="scps", name="oproj1")
                rps8 = sc_ps.tile([128, 1024], F32, tag="scps", name="rps8")
            for g in range(8):
                if p < 8:
                    # psA holds skv-chunk 2g for BOTH heads (h0 cols 0:512,
                    # h1 cols 512:1024): the two fill matmuls use different
                    # PE row groups, so they run concurrently
                    psA = sc_ps.tile([128, 1024], F32, tag="scps")
                    psB = sc_ps.tile([128, 1024], F32, tag="scps")
                    for tt, pst in ((0, psA), (1, psB)):
                        t = g * 2 + tt
                        nc.tensor.matmul(
                            pst[:, 0:512],
                            vt[p][0:64, t * 128 : (t + 1) * 128],
                            qe[0:64, :],
                            start=True, stop=True, tile_position=(0, 0),
                        )
                        nc.tensor.matmul(
                            pst[:, 512:1024],
                            vt[p][64:128, t * 128 : (t + 1) * 128],
                            qe[64:128, :],
                            start=True, stop=True, tile_position=(64, 0),
                        )
                if p > 0:
                    for t in (2 * g, 2 * g + 1):
                        u_mm(p - 1, ups_cur, Eprev, t)
                if p == 8 and g < 7:
                    # normalize pair g; rps double-buffered by column half so
                    # sel(g) only waits on mul(g-2)
                    half = (g % 2) * 512
                    nc.tensor.matmul(
                        rps8[:, half : half + 512],
                        sel14_s[:, g * 128 : (g + 1) * 128],
                        rr14[:],
                        start=True, stop=True,
                    )
                    ot = outp.tile(
                        [128, SQB], BF16, tag=f"outT{g}", name=f"ot{g}"
                    )
                    nc.vector.tensor_mul(ot[:], U2[g][:], rps8[:, half : half + 512])
                    outT[g] = ot
                if p == 8 and 1 <= g <= 6:
                    # output projection m=0/1, contraction chunk kk=g-1
                    kk = g - 1
                    for m in range(2):
                        for n in range(2):
                            nc.tensor.matmul(
                                oproj_ps[m][:, n * 512 : (n + 1) * 512],
                                outT[kk][:, m * 128 : (m + 1) * 128],
                                wd_s[kk][:, n * 512 : (n + 1) * 512],
                                start=(kk == 0), stop=False,
                                skip_group_check=True,
                            )
                if p < 8:
                    ea = ep.tile([128, 1024], BF16, tag="E", name=f"ea{p}_{g}")
                    nc.scalar.activation(ea[:], psA[:], EXP, scale=0.125)
                    Ech.append(ea)
                    eb = ep.tile([128, 1024], BF16, tag="E", name=f"eb{p}_{g}")
                    if g < ACT_B:
                        nc.scalar.activation(eb[:], psB[:], EXP, scale=0.125)
                    else:
                        nc.vector.tensor_scalar(
                            eb[:].bitcast(I16), psB[:], SCH_A, SCH_B,
                            op0=MULT, op1=ADD,
                        )
                    Ech.append(eb)
            if p > 0:
                evac(p - 1, ups_cur)
                ups_prev = ups_cur
            Eprev = Ech

        # ---- tail: kk=6 for m0/m1, normalize pair 7, kk=7, then m2/m3 ----
        ln7 = wpool.tile([2, SQB], F32, tag="ln7")
        nc.scalar.activation(ln7[:], rg7[:], mybir.ActivationFunctionType.Ln)
        with nc.allow_low_precision(reason="exp(-ln r): 2-ULP spline, plenty for softmax denom"):
            nc.scalar.activation(rr7[:], ln7[:], EXP, scale=-1.0)
        for m in range(2):
            for n in range(2):
                nc.tensor.matmul(
                    oproj_ps[m][:, n * 512 : (n + 1) * 512],
                    outT[6][:, m * 128 : (m + 1) * 128],
                    wd_s[6][:, n * 512 : (n + 1) * 512],
                    start=False, stop=False, skip_group_check=True,
                )
        nc.tensor.matmul(rps8[:, 512:1024], sel2_s[:], rr7[:], start=True, stop=True)
        ot7 = outp.tile([128, SQB], BF16, tag="outT7")
        nc.vector.tensor_mul(ot7[:], U2[7][:], rps8[:, 512:1024])
        outT[7] = ot7
        for m in range(2):
            for n in range(2):
                nc.tensor.matmul(
                    oproj_ps[m][:, n * 512 : (n + 1) * 512],
                    outT[7][:, m * 128 : (m + 1) * 128],
                    wd_s[7][:, n * 512 : (n + 1) * 512],
                    start=False, stop=True, skip_group_check=True,
                )
            osb = osbp.tile([128, E], BF16, tag="osb")
            nc.vector.tensor_add(osb[:], oproj_ps[m][:], bd_s[:])
            nc.sync.dma_start(out[m * 128 : (m + 1) * 128, :], osb[:])
        for m in range(2, 4):
            oproj_ps[m] = sc_ps.tile([128, 1024], F32, tag="scps", name=f"oproj{m}")
            for n in range(2):
                for kk in range(8):
                    nc.tensor.matmul(
                        oproj_ps[m][:, n * 512 : (n + 1) * 512],
                        outT[kk][:, m * 128 : (m + 1) * 128],
                        wd_s[kk][:, n * 512 : (n + 1) * 512],
                        start=(kk == 0), stop=(kk == 7),
                    )
            osb = osbp.tile([128, E], BF16, tag="osb")
            nc.vector.tensor_add(osb[:], oproj_ps[m][:], bd_s[:])
            nc.sync.dma_start(out[m * 128 : (m + 1) * 128, :], osb[:])

    nc.compile()
    return nc


def kernel(queries, values, heads, Wv, bv, Wk, bk, Wq, bq, Wd, bd, **_):
    queries = np.asarray(queries, np.float32)
    values = np.asarray(values, np.float32)
    Wv, bv = np.asarray(Wv, np.float64), np.asarray(bv, np.float64)
    Wk = np.asarray(Wk, np.float64)
    Wq, bq = np.asarray(Wq, np.float64), np.asarray(bq, np.float64)
    Wd, bd = np.asarray(Wd, np.float64), np.asarray(bd, np.float64)
    assert int(heads) == H and queries.shape == (B, S, E)

    if "nc" not in _CACHE:
        _CACHE["nc"] = _build_program()
    nc = _CACHE["nc"]

    def blk(A):
        Z = np.zeros_like(A)
        return np.block([[A, Z], [Z, A]]).astype(NP_BF16)

    Wkv = Wk @ Wv
    wq2 = blk(Wq.T @ Wkv)                       # lhsT for qe projection
    cq2 = np.tile(Wkv.T @ bq, 2)[:, None].astype(np.float32)
    sel14 = np.zeros((14, 7 * 128), np.float32)
    for p in range(7):
        for m in range(128):
            sel14[2 * p + m // 64, p * 128 + m] = 1.0
    sel2 = np.zeros((2, 128), np.float32)
    sel2[0, 0:64] = 1.0
    sel2[1, 64:128] = 1.0
    bv_full = np.tile(bv, H)
    bd_rep = np.tile((bd + Wd @ bv_full)[None, :], (128, 1)).astype(np.float32)
    Vblk = np.zeros((E, E))
    for h in range(H):
        Vblk[h * D : (h + 1) * D, h * D : (h + 1) * D] = Wv
    wdT = np.ascontiguousarray((Wd @ Vblk).T).astype(NP_BF16)

    vT_b, vnat_b = [], []
    for b_ in range(B):
        vT_b.append(np.ascontiguousarray(values[b_].T).astype(NP_BF16))
        vn = np.zeros((S, H * 128), np.float32)
        vr = values[b_].reshape(S, H, D)
        for h in range(H):
            vn[:, h * 128 : h * 128 + 64] = vr[:, h, :]
            vn[:, h * 128 + 64] = 1.0
        vnat_b.append(vn.astype(NP_BF16))

    common = dict(wq2=wq2, cq2=cq2, sel14=sel14, sel2=sel2, wdT=wdT,
                  bd_rep=bd_rep)
    in_maps = []
    for c in range(N_CORES):
        b_, qb = c // 4, c % 4
        in_maps.append(dict(
            qT_in=np.ascontiguousarray(
                queries[b_, qb * SQB : (qb + 1) * SQB, :].T
            ).astype(NP_BF16),
            vT_in=vT_b[b_],
            vnat_in=vnat_b[b_],
            **common,
        ))

    _CACHE["last_in_maps"] = in_maps
    res = run_bass_kernel_spmd(nc, in_maps, list(range(N_CORES)))
    out = np.empty((B, S, E), np.float32)
    for c in range(N_CORES):
        b_, qb = c // 4, c % 4
        out[b_, qb * SQB : (qb + 1) * SQB, :] = res.results[c]["out"].astype(np.float32)
    return out
